# revision 16
# baseline (speedup 1.0000x reference)
"""3-layer GAT on 8 trn2 NeuronCores (Bass/Tile).

Sharding: destination nodes block-sharded npc=N/8 per core; each core owns the
edges into its nodes, grouped by 128-dst-node "groups". Segment softmax +
neighbor aggregation are per-group PSUM matmuls with on-chip one-hot selection
matrices scaled by exp(attention). Source features are fetched with dma_gather
(int16 tokens) from chunked node tables; layer-0 tables are host-baked, later
layers AllGather dense projections in 3 chunks issued on the SP/Act/PE queues
(keeping the Pool/SWDGE queue free for gathers).

Self-contained: host preprocessing + Bass program + execution.
"""
import sys
import numpy as np

sys.path.insert(0, "/opt/trn_rl_repo")

import concourse.bass as bass  # noqa: E402
import concourse.bacc as bacc  # noqa: E402
import concourse.tile as tile  # noqa: E402
from concourse import mybir  # noqa: E402
from concourse.bass_utils import run_bass_kernel_spmd  # noqa: E402
from concourse.masks import make_identity  # noqa: E402

dt = mybir.dt
AF = mybir.ActivationFunctionType
ALU = mybir.AluOpType

NEG_SLOPE = 0.2
P = 128
NCHUNK = 3
GROUPS_PER_WIN = 3


def _bf16(x):
    import ml_dtypes
    return np.asarray(x).astype(ml_dtypes.bfloat16)


# ---------------------------------------------------------------- host plan

class Plan:
    pass


def build_plan(N, src_all, dst_all, ncores):
    """Static structure shared by all cores (token counts use max over cores).

    src/dst include self loops (int64)."""
    pl = Plan()
    pl.N, pl.ncores = N, ncores
    assert N % ncores == 0
    pl.npc = N // ncores
    ngroups = (pl.npc + P - 1) // P
    pl.ngroups = ngroups
    pl.nrows_grp = [min(P, pl.npc - g * P) for g in range(ngroups)]

    # node id == table row; int16 gather tokens are relative to hsplit halves
    pl.hsplit = ((N // 2) // P) * P + P
    pl.span = [pl.hsplit, N - pl.hsplit]
    assert all(sp < 32768 for sp in pl.span)
    n_ids = np.arange(N, dtype=np.int64)
    chunk_of = (n_ids >= pl.hsplit).astype(np.int64)
    row_of = n_ids - chunk_of * pl.hsplit
    pl.chunk_of, pl.row_of = chunk_of, row_of

    order = np.argsort(dst_all, kind="stable")
    s_sorted, d_sorted = src_all[order], dst_all[order]

    # per (core, group, chunk): token rows + local dst
    per = [[[None] * NCHUNK for _ in range(ngroups)] for _ in range(ncores)]
    for m in range(ncores):
        lo = np.searchsorted(d_sorted, m * pl.npc, side="left")
        hi = np.searchsorted(d_sorted, (m + 1) * pl.npc - 1, side="right")
        s_e = s_sorted[lo:hi]
        dloc_e = d_sorted[lo:hi] - m * pl.npc
        gid = dloc_e // P
        s_row = row_of[s_e]
        s_chunk = chunk_of[s_e]
        for g in range(ngroups):
            gm = gid == g
            for c in range(NCHUNK):
                mask = gm & (s_chunk == c)
                per[m][g][c] = [s_row[mask], dloc_e[mask] % P]

    # fake edges so pad rows of the last group have nonzero denominators
    lastg = ngroups - 1
    nfake = ngroups * P - pl.npc
    if nfake:
        for m in range(ncores):
            sg, dg = per[m][lastg][0]
            per[m][lastg][0] = [
                np.concatenate([sg, np.zeros(nfake, sg.dtype)]),
                np.concatenate([dg, np.arange(pl.nrows_grp[lastg], P,
                                              dtype=dg.dtype)]),
            ]

    # tiles per (group, chunk) from the max token count over cores
    pl.maxlen = np.zeros((ngroups, NCHUNK), np.int64)
    for g in range(ngroups):
        for c in range(NCHUNK):
            pl.maxlen[g, c] = max(len(per[m][g][c][0]) for m in range(ncores))
    pl.kgc = (pl.maxlen + P - 1) // P
    for g in range(ngroups):
        if pl.kgc[g].sum() == 0:
            pl.kgc[g, 0] = 1
            pl.maxlen[g, 0] = 1
    # full-fill gathers: every slab slot is written (pad tokens hit row 0),
    # so no slot ever holds uninitialized SBUF
    pl.maxlen = pl.kgc * P
    pl.kg = pl.kgc.sum(axis=1)
    pl.gbase = np.concatenate([[0], np.cumsum(pl.kg)[:-1]])
    TT = int(pl.kg.sum())
    pl.TT = TT

    # token-stream column offsets (16 tokens per packed column)
    pl.col0 = np.zeros((ngroups, NCHUNK), np.int64)
    cur = 0
    for g in range(ngroups):
        for c in range(NCHUNK):
            pl.col0[g, c] = cur
            cur += (int(pl.maxlen[g, c]) + 15) // 16
    pl.src_cols = max(cur, 1)

    # per-core per-slot srcidx / dloc
    pl.srcidx = np.zeros((ncores, TT, P), np.int64)
    pl.dloc = np.full((ncores, TT, P), -1.0, np.float32)
    for m in range(ncores):
        for g in range(ngroups):
            t0 = int(pl.gbase[g])
            for c in range(NCHUNK):
                sg, dg = per[m][g][c]
                base_t = t0 + int(pl.kgc[g, :c].sum())
                for k in range(int(pl.kgc[g, c])):
                    a, b = k * P, min((k + 1) * P, len(sg))
                    if b > a:
                        pl.srcidx[m, base_t + k, : b - a] = sg[a:b]
                        pl.dloc[m, base_t + k, : b - a] = dg[a:b]

    # windows
    pl.windows = []
    g = 0
    while g < ngroups:
        gw = list(range(g, min(g + GROUPS_PER_WIN, ngroups)))
        t0 = int(pl.gbase[gw[0]])
        nwt = int(sum(pl.kg[gg] for gg in gw))
        pl.windows.append({"groups": gw, "t0": t0, "nwt": nwt})
        g += GROUPS_PER_WIN

    def pack(tok_cols):
        """tok_cols: int16 array [16, ncol] -> [128, ncol] (replicated x8)."""
        return np.tile(tok_cols, (8, 1))

    # src token table [P, src_cols] per core
    pl.idx_packed = []
    pl.idxd_packed = []
    for m in range(ncores):
        blk = np.zeros((16, pl.src_cols), np.int16)
        for g in range(ngroups):
            for c in range(NCHUNK):
                ml = int(pl.maxlen[g, c])
                if ml == 0:
                    continue
                t0 = int(pl.gbase[g] + pl.kgc[g, :c].sum())
                toks = pl.srcidx[m, t0:t0 + int(pl.kgc[g, c])].reshape(-1)[:ml]
                co = int(pl.col0[g, c])
                idx = np.arange(ml)
                blk[idx % 16, co + idx // 16] = toks.astype(np.int16)
        pl.idx_packed.append(pack(blk))
        # dst tokens, slot-major: local adst-table row = g*128 + dst_local
        dt_toks = np.zeros(TT * P, np.int64)
        for g in range(ngroups):
            t0 = int(pl.gbase[g])
            for t in range(t0, t0 + int(pl.kg[g])):
                d = pl.dloc[m, t]
                dt_toks[t * P:(t + 1) * P] = np.where(
                    d >= 0, g * P + np.maximum(d, 0), 0)
        blkd = np.zeros((16, TT * 8), np.int16)
        idx = np.arange(TT * P)
        blkd[idx % 16, idx // 16] = dt_toks.astype(np.int16)
        pl.idxd_packed.append(pack(blkd))
    return pl


# ---------------------------------------------------------------- builder

def build_program(pl, HID, C, scratch=65536):
    ncores, TT, ngroups, npc = pl.ncores, pl.TT, pl.ngroups, pl.npc
    EW = [256, 256, 128]          # gathered elems per src token
    HC = [HID, HID, C]            # h width of the table feeding each layer
    # table col layout per layer: [h(HC) | one | asrc]; rhs = cols 0..HC
    DOUT = [HID, HID, C]

    nc = bacc.Bacc(None, num_devices=ncores, dynamic_dma_scratch_size=scratch)

    N = pl.N
    t0_in = nc.declare_dram_parameter("t0", [N, 256], dt.bfloat16,
                                      isOutput=False)
    adst0_in = nc.declare_dram_parameter("adst0", [ngroups * P, 128],
                                         dt.bfloat16, isOutput=False)
    dloc_in = nc.declare_dram_parameter("dloc", [P, TT], dt.float32,
                                        isOutput=False)
    iota_in = nc.declare_dram_parameter("iota", [P, P], dt.bfloat16,
                                        isOutput=False)
    idx_in = nc.declare_dram_parameter("idx_src", [P, pl.src_cols], dt.int16,
                                       isOutput=False)
    idxd_in = nc.declare_dram_parameter("idx_dst", [P, TT * 8], dt.int16,
                                        isOutput=False)
    waug1_in = nc.declare_dram_parameter("waug1", [HID, HID + 2], dt.bfloat16,
                                         isOutput=False)
    waug2_in = nc.declare_dram_parameter("waug2", [HID, C + 2], dt.bfloat16,
                                         isOutput=False)
    bias_in = nc.declare_dram_parameter("bias", [P, 3 * HID], dt.float32,
                                        isOutput=False)
    out_p = nc.declare_dram_parameter("out", [npc, C], dt.float32,
                                      isOutput=True)

    # cc staging (own rows) -> compact AllGather -> re-strided gather table;
    # payload cols = h|one|asrc
    CCC = [HID + 2, C + 2]
    cc_in = [nc.dram_tensor(f"cc{b}", [npc, CCC[b]], dt.bfloat16)
             for b in range(2)]
    cmp_t = [nc.dram_tensor(f"cmp{b}", [N, CCC[b]], dt.bfloat16,
                            addr_space="Shared") for b in range(2)]
    tbl = [nc.dram_tensor(f"tbl{b}", [N, 256], dt.bfloat16)
           for b in range(2)]
    adstA = nc.dram_tensor("adstA", [ngroups * P, 128], dt.bfloat16)
    adstB = nc.dram_tensor("adstB", [ngroups * P, 128], dt.bfloat16)
    adst_tbls = [adst0_in, adstA, adstB]
    tables = [t0_in] + tbl

    rg = [list(range(ncores))]
    cc_engines = [nc.gpsimd, nc.gpsimd, nc.gpsimd]

    def cc_on(eng, in_ap, out_ap):
        eng.bass.has_collectives = True
        return eng.add_instruction(
            mybir.InstCollectiveCompute(
                name=f"I-{eng.bass.next_id()}",
                kind="AllGather",
                op=ALU.bypass,
                replica_groups=rg,
                ins=[eng.lower_ap(in_ap)],
                outs=[eng.lower_ap(out_ap)],
                unique_tensors="No",
                cc_dim="Partition",
            ))

    ntok_regs = {}

    def reg_of(n):
        if n not in ntok_regs:
            ntok_regs[n] = nc.gpsimd.to_reg(n)
        return ntok_regs[n]

    with tile.TileContext(nc) as tc:
        with (
            tc.tile_pool(name="res", bufs=1) as res,
            tc.tile_pool(name="slab", bufs=2) as slab_pool,
            tc.tile_pool(name="selw", bufs=2) as selw_pool,
            tc.tile_pool(name="adv", bufs=len(pl.windows)) as adv_pool,
            tc.tile_pool(name="sel", bufs=16) as sel_pool,
            tc.tile_pool(name="grp", bufs=4) as grp_pool,
            tc.tile_pool(name="eplg", bufs=4) as ep_pool,
            tc.tile_pool(name="ps_agg", bufs=3, space="PSUM") as ps_agg,
            tc.tile_pool(name="ps_dense", bufs=2, space="PSUM") as ps_dense,
            tc.tile_pool(name="ps_tr", bufs=2, space="PSUM") as ps_tr,
        ):
            iota_t = res.tile([P, P], dt.bfloat16)
            nc.sync.dma_start(out=iota_t[:], in_=iota_in[:, :])
            dloc_t = res.tile([P, TT], dt.float32)
            nc.sync.dma_start(out=dloc_t[:], in_=dloc_in[:, :])
            idx_t = res.tile([P, pl.src_cols], dt.int16, name="idxs")
            nc.sync.dma_start(out=idx_t[:], in_=idx_in[:, :])
            idxd_t = res.tile([P, TT * 8], dt.int16, name="idxd")
            nc.sync.dma_start(out=idxd_t[:], in_=idxd_in[:, :])
            waug_t = [None, res.tile([HID, HID + 2], dt.bfloat16, name="waug1"),
                      res.tile([HID, C + 2], dt.bfloat16, name="waug2")]
            nc.sync.dma_start(out=waug_t[1][:], in_=waug1_in[:, :])
            nc.sync.dma_start(out=waug_t[2][:], in_=waug2_in[:, :])
            bias_t = res.tile([P, 3 * HID], dt.float32)
            nc.sync.dma_start(out=bias_t[:], in_=bias_in[:, :])
            xT_own = res.tile([P, ngroups * P], dt.bfloat16)
            ident = res.tile([P, P], dt.bfloat16)
            make_identity(nc, ident[:])
            ones_t = res.tile([P, 64], dt.bfloat16, name="ones")
            nc.vector.memset(ones_t[:], 1.0)
            hv_all = res.tile([P, ngroups * C], dt.float32, name="hvall")
            mx_all = res.tile([P, ngroups + 1], dt.float32, name="mxall")
            sm_all = res.tile([P, ngroups + 1], dt.float32, name="small")

            # ones column of cc staging (constant across the run)
            for b in range(2):
                onec = HC[b + 1]
                full = npc // P
                if full:
                    nc.scalar.dma_start(
                        out=cc_in[b][0:full * P, onec:onec + 1],
                        in_=ones_t[:, 0:full])
                rem = npc - full * P
                if rem:
                    nc.scalar.dma_start(
                        out=cc_in[b][full * P:npc, onec:onec + 1],
                        in_=ones_t[0:rem, full:full + 1])
            # zero adst tables (gather input must be finite)
            z = res.tile([P, 128], dt.bfloat16, name="z")
            nc.vector.memset(z[:], 0.0)
            for tb in (adstA, adstB):
                for g0 in range(ngroups):
                    nc.scalar.dma_start(out=tb[g0 * P:(g0 + 1) * P, :],
                                        in_=z[:])

            for lyr in range(3):
                TBL = tables[lyr]
                ATBL = adst_tbls[lyr]
                ew, hc, dout = EW[lyr], HC[lyr], DOUT[lyr]

                # dst-side adst gathers for the whole layer first (they only
                # need local data, so they overlap the previous AllGather)
                adv_tiles = []
                for wi, w in enumerate(pl.windows):
                    nwt, t0w = w["nwt"], w["t0"]
                    slab_d = selw_pool.tile([P, nwt * P], dt.bfloat16,
                                            name="slabd")
                    ntok = nwt * P
                    out_ap = bass.AP(slab_d[:].tensor, slab_d[:].offset,
                                     [slab_d[:].ap[0], [P, nwt], [1, P]])
                    nc.gpsimd.dma_gather(
                        out_ap=out_ap, in_ap=ATBL[:, :],
                        idxs_ap=idxd_t[:, t0w * 8:(t0w + nwt) * 8],
                        num_idxs=ntok, num_idxs_reg=reg_of(ntok),
                        elem_size=P, elem_step=P)
                    av = adv_pool.tile([P, nwt], dt.bfloat16, name="adv")
                    src_ap = bass.AP(slab_d[:].tensor, slab_d[:].offset,
                                     [slab_d[:].ap[0], [P, nwt]])
                    nc.vector.tensor_copy(out=av[:], in_=src_ap)
                    adv_tiles.append(av)

                for wi, w in enumerate(pl.windows):
                    nwt, t0w = w["nwt"], w["t0"]
                    av = adv_tiles[wi]
                    slab = slab_pool.tile([P, nwt * ew], dt.bfloat16,
                                          name="slab")

                    # src gathers per (group, chunk)
                    for g in w["groups"]:
                        for c in range(NCHUNK):
                            ml = int(pl.maxlen[g, c])
                            if ml == 0:
                                continue
                            kk = int(pl.kgc[g, c])
                            tb = int(pl.gbase[g] + pl.kgc[g, :c].sum()) - t0w
                            out_ap = bass.AP(
                                slab[:].tensor, slab[:].offset + tb * ew,
                                [slab[:].ap[0], [ew, kk], [1, ew]])
                            in_ap = bass.AP(
                                TBL[:, :].tensor, c * pl.hsplit * 256,
                                [[256, pl.span[c]], [1, ew]])
                            co = int(pl.col0[g, c])
                            ncol = (ml + 15) // 16
                            nc.gpsimd.dma_gather(
                                out_ap=out_ap, in_ap=in_ap,
                                idxs_ap=idx_t[:, co:co + ncol],
                                num_idxs=ml, num_idxs_reg=reg_of(ml),
                                elem_size=ew, elem_step=256)

                    for g in w["groups"]:
                        kg = int(pl.kg[g])
                        i0 = int(pl.gbase[g]) - t0w
                        nrow = pl.nrows_grp[g]

                        al_t = grp_pool.tile([P, max(kg, 2)], dt.float32,
                                             name="al")
                        ex_t = grp_pool.tile([P, max(kg, 2)], dt.float32,
                                             name="ex")
                        s0 = 0
                        for c in range(NCHUNK):
                            kk = int(pl.kgc[g, c])
                            if kk == 0:
                                continue
                            asrc_view = bass.AP(
                                slab[:].tensor,
                                slab[:].offset + (i0 + s0) * ew + hc + 1,
                                [slab[:].ap[0], [ew, kk]])
                            nc.vector.tensor_tensor(
                                out=al_t[:, s0:s0 + kk], in0=asrc_view,
                                in1=av[:, i0 + s0:i0 + s0 + kk], op=ALU.add)
                            s0 += kk
                        nc.vector.tensor_scalar(
                            out=ex_t[:, 0:kg], in0=al_t[:, 0:kg],
                            scalar1=NEG_SLOPE, scalar2=None, op0=ALU.mult)
                        nc.vector.tensor_tensor(
                            out=ex_t[:, 0:kg], in0=ex_t[:, 0:kg],
                            in1=al_t[:, 0:kg], op=ALU.max)
                        nc.scalar.activation(ex_t[:, 0:kg], ex_t[:, 0:kg],
                                             AF.Exp)

                        agg_ps = ps_agg.tile([P, hc + 1], dt.float32,
                                             space="PSUM", name="agg")
                        for i in range(kg):
                            t = int(pl.gbase[g]) + i
                            rhs = bass.AP(slab[:].tensor,
                                          slab[:].offset + (i0 + i) * ew,
                                          [slab[:].ap[0], [1, hc + 1]])
                            selp = sel_pool.tile([P, P], dt.bfloat16,
                                                 name="selp")
                            nc.vector.tensor_scalar(
                                out=selp[:], in0=iota_t[:],
                                scalar1=dloc_t[:, t:t + 1],
                                scalar2=ex_t[:, i:i + 1],
                                op0=ALU.is_equal, op1=ALU.mult)
                            nc.tensor.matmul(agg_ps[:], lhsT=selp[:], rhs=rhs,
                                             start=(i == 0), stop=(i == kg - 1))

                        recip = ep_pool.tile([P, 1], dt.float32, name="recip")
                        nc.vector.reciprocal(recip[:], agg_ps[:, hc:hc + 1])
                        hv = ep_pool.tile([P, dout], dt.float32, name="hv")
                        nc.vector.tensor_scalar(
                            out=hv[:], in0=agg_ps[:, 0:dout],
                            scalar1=recip[:, 0:1], scalar2=None, op0=ALU.mult)
                        nc.vector.tensor_tensor(
                            out=hv[:], in0=hv[:],
                            in1=bias_t[:, lyr * HID:lyr * HID + dout],
                            op=ALU.add)
                        if lyr < 2:
                            # silu via exp (keeps Act on the Exp/Ln table)
                            ev = ep_pool.tile([P, dout], dt.float32, name="ev")
                            nc.scalar.activation(ev[:], hv[:], AF.Exp,
                                                 scale=-1.0)
                            nc.vector.tensor_scalar(
                                out=ev[:], in0=ev[:], scalar1=1.0,
                                scalar2=None, op0=ALU.add)
                            nc.vector.reciprocal(ev[:], ev[:])
                            xn = ep_pool.tile([P, dout], dt.bfloat16,
                                              name="xn")
                            nc.vector.tensor_tensor(out=xn[:], in0=hv[:],
                                                    in1=ev[:], op=ALU.mult)
                            tr_ps = ps_tr.tile([P, P], dt.bfloat16,
                                               space="PSUM", name="tr")
                            nc.tensor.transpose(tr_ps[:], xn[:], ident[:])
                            nc.vector.tensor_copy(
                                out=xT_own[:, g * P:(g + 1) * P], in_=tr_ps[:])
                            nl = lyr + 1
                            hcn = HC[nl]
                            dn_ps = ps_dense.tile([P, hcn + 2], dt.float32,
                                                  space="PSUM", name="dn")
                            nc.tensor.matmul(dn_ps[0:nrow, :],
                                             lhsT=xT_own[:, g * P:g * P + nrow],
                                             rhs=waug_t[nl][:],
                                             start=True, stop=True)
                            row = ep_pool.tile([P, hcn + 2], dt.bfloat16,
                                               name="row")
                            nc.vector.tensor_copy(out=row[0:nrow, :],
                                                  in_=dn_ps[0:nrow, :])
                            r0 = g * P
                            cci = cc_in[lyr]
                            nc.sync.dma_start(
                                out=cci[r0:r0 + nrow, 0:hcn],
                                in_=row[0:nrow, 0:hcn])
                            nc.sync.dma_start(
                                out=cci[r0:r0 + nrow, hcn + 1:hcn + 2],
                                in_=row[0:nrow, hcn:hcn + 1])
                            nxt_a = adstA if lyr == 0 else adstB
                            nc.sync.dma_start(
                                out=nxt_a[g * P:g * P + nrow, 0:1],
                                in_=row[0:nrow, hcn + 1:hcn + 2])
                        else:
                            # stash hv/max/sumexp; one batched Ln at the end
                            # keeps the Act engine on the Exp table all layer
                            hvg = bass.AP(hv_all[:].tensor,
                                          hv_all[:].offset + g * C,
                                          [hv_all[:].ap[0], [1, C]])
                            nc.vector.tensor_copy(out=hvg, in_=hv[:])
                            nc.vector.reduce_max(mx_all[:, g:g + 1], hv[:],
                                                 axis=mybir.AxisListType.X,
                                                 negate=True)
                            ev = ep_pool.tile([P, dout], dt.float32, name="ev")
                            nc.scalar.activation(ev[:], hv[:], AF.Exp,
                                                 bias=mx_all[:, g:g + 1])
                            nc.vector.reduce_sum(sm_all[:, g:g + 1], ev[:],
                                                 axis=mybir.AxisListType.X)

                if lyr == 2:
                    lns = res.tile([P, ngroups + 1], dt.float32, name="lns")
                    nc.scalar.activation(lns[:, 0:ngroups],
                                         sm_all[:, 0:ngroups], AF.Ln)
                    for g in range(ngroups):
                        nrow = pl.nrows_grp[g]
                        o_sb = ep_pool.tile([P, C], dt.float32, name="ou")
                        hvg = bass.AP(hv_all[:].tensor,
                                      hv_all[:].offset + g * C,
                                      [hv_all[:].ap[0], [1, C]])
                        nc.vector.tensor_scalar(
                            out=o_sb[:], in0=hvg,
                            scalar1=mx_all[:, g:g + 1],
                            scalar2=lns[:, g:g + 1],
                            op0=ALU.add, op1=ALU.subtract)
                        nc.sync.dma_start(out=out_p[g * P:g * P + nrow, :],
                                          in_=o_sb[0:nrow, :])

                if lyr < 2:
                    ccc = CCC[lyr]
                    cc_on(nc.gpsimd, cc_in[lyr][0:npc, 0:ccc],
                          cmp_t[lyr][0:N, 0:ccc])
                    nc.sync.dma_start(out=tbl[lyr][0:N, 0:ccc],
                                      in_=cmp_t[lyr][0:N, 0:ccc])
    nc.compile()
    return nc


# ---------------------------------------------------------------- host side

def make_inputs(pl, x, W, a_s, a_d, b, HID, C):
    """Per-core in_maps. W/a_s/a_d/b: lists of 3 arrays."""
    N, ncores, ngroups, npc = pl.N, pl.ncores, pl.ngroups, pl.npc
    waug = []
    for l in range(3):
        waug.append(np.concatenate(
            [W[l], (W[l] @ a_s[l])[:, None], (W[l] @ a_d[l])[:, None]],
            axis=1).astype(np.float32))

    # layer-0 table host-baked: row == node id, cols [h | one | asrc]
    h0 = x.astype(np.float32) @ waug[0]          # [N, HID+2]
    t0 = np.zeros((N, 256), np.float32)
    t0[:, :HID] = h0[:, :HID]
    t0[:, HID] = 1.0
    t0[:, HID + 1] = h0[:, HID]                  # asrc
    t0 = _bf16(t0)

    iota = np.broadcast_to(np.arange(P, dtype=np.float32)[None, :],
                           (P, P)).copy()
    bias = np.zeros((P, 3 * HID), np.float32)
    bias[:, 0 * HID:0 * HID + HID] = b[0][None, :]
    bias[:, 1 * HID:1 * HID + HID] = b[1][None, :]
    bias[:, 2 * HID:2 * HID + C] = b[2][None, :]

    in_maps = []
    for m in range(ncores):
        adst0 = np.zeros((ngroups * P, 128), np.float32)
        adst0[:npc, 0] = h0[m * npc:(m + 1) * npc, HID + 1]
        im = dict(
            dloc=pl.dloc[m].T.copy().astype(np.float32).reshape(P, pl.TT),
            iota=_bf16(iota),
            idx_src=pl.idx_packed[m],
            idx_dst=pl.idxd_packed[m],
            waug1=_bf16(waug[1]),
            waug2=_bf16(waug[2]),
            adst0=_bf16(adst0),
            bias=bias,
        )
        im["t0"] = t0
        in_maps.append(im)
    return in_maps


_CACHE = {}


def _get_program(key, pl, HID, C):
    if key not in _CACHE:
        _CACHE[key] = build_program(pl, HID, C)
    return _CACHE[key]


def gat_forward(x, edge_index, W, a_s, a_d, b, ncores=8):
    N = x.shape[0]
    HID = W[0].shape[1]
    C = W[2].shape[1]
    loops = np.arange(N, dtype=np.int64)
    src = np.concatenate([np.asarray(edge_index[0], np.int64), loops])
    dst = np.concatenate([np.asarray(edge_index[1], np.int64), loops])
    pl = build_plan(N, src, dst, ncores)
    nc = _get_program((N, len(src), ncores, HID, C), pl, HID, C)
    in_maps = make_inputs(pl, np.asarray(x), W, a_s, a_d, b, HID, C)
    res = run_bass_kernel_spmd(nc, in_maps, core_ids=list(range(ncores)))
    out = np.concatenate([np.asarray(res.results[m]["out"])
                          for m in range(ncores)], axis=0)
    return out.astype(np.float32)


def kernel(x, edge_index, W0, a_src0, a_dst0, b0, W1, a_src1, a_dst1, b1,
           W2, a_src2, a_dst2, b2):
    f32 = lambda t: np.asarray(t, dtype=np.float32)
    return gat_forward(
        f32(x), np.asarray(edge_index),
        [f32(W0), f32(W1), f32(W2)],
        [f32(a_src0), f32(a_src1), f32(a_src2)],
        [f32(a_dst0), f32(a_dst1), f32(a_dst2)],
        [f32(b0), f32(b1), f32(b2)],
    )
